# revision 2
# baseline (speedup 1.0000x reference)
"""Trainium2 Bass kernel for nn_BlockCorrelation — v2 (locality rewrite).

Scheme (vs the head-parallel baseline):
  - rows are sorted by group; core c owns the contiguous sorted range
    [c*256, (c+1)*256).  Since every group has <=128 rows, all keys a core's
    queries attend to lie in a +-128-row halo around its range.
  - each core pools + LayerNorms its own rows (x streamed once in fp8,
    CCE-accumulate folds 64->16 spatially during the DMA, DVE folds the rest),
    computes q/k/v for its own rows with FULL heads (no tensor parallelism),
    AllGathers the tiny LN'd feature tiles, reads its two 128-row halos from
    the gathered table with dynamic (partition_id-based) DMA offsets, runs
    block-masked attention locally, and streams y = x + g*deltaT with the
    broadcast on a middle dim so the DVE add runs in 2x mode.
  - NO ReduceScatter; the only cross-core syncs are two 0.25 MB AllGathers
    (one per 128-row tile, pipelined with pooling of the other tile).
  - x layout is [cb, ch, hw, row] so pool folds, the gd broadcast add, and
    all DMA descriptors (>=2KB contiguous per partition) are simultaneously
    efficient.
  - the block mask is added inside the scores PSUM via the rank-32 one-hot
    matmul; absent halos (edge cores) are memset to zero and masked via
    all-ones -50 one-hot columns (host data), so exp() gives exactly 0.
"""

import json
import sys

if "/opt/trn_rl_repo" not in sys.path:
    sys.path.insert(0, "/opt/trn_rl_repo")

import ml_dtypes
import numpy as np

import concourse.bass as bass
import concourse.mybir as mybir
import concourse.tile as tile
from concourse.bass_utils import run_bass_kernel_spmd

F32 = mybir.dt.float32
BF16 = mybir.dt.bfloat16
FP8 = mybir.dt.float8e4

N, C, HW = 2048, 1024, 64
NH, HD = 8, 128
NG = 32
EPS = 1e-5
NCORES = 8
NS = N // NCORES          # 256 rows per core
CB = C // 128             # 8 channel blocks
RT = 2                    # 128-row tiles per core
WIN = 512                 # key window: [own0, own1, haloL, haloR]
GCAP = 128
MASK_NEG = -50.0
POOL_SUB = 4              # CCE-accum sub-DMAs folding hw 64 -> 16
XAB = 7                   # add-pass x prefetch buffers


def _split_waits_json(j, max_waits=1):
    for f in j.get("functions", []):
        for bb in f.get("blocks", []):
            out = []
            for ins in bb.get("instructions", []):
                si = ins.get("sync_info")
                waits = (si or {}).get("on_wait") or []
                if len(waits) > max_waits:
                    head, tail = waits[:-max_waits], waits[-max_waits:]
                    for k, w in enumerate(head):
                        out.append({
                            "name": f"{ins['name']}-wsplit{k}",
                            "opcode": "EventSemaphore",
                            "engine": ins["engine"],
                            "ins": [],
                            "outs": [],
                            "debug": ins.get("debug", 0),
                            "sync_info": {"on_update": [], "on_wait": [w]},
                        })
                    si["on_wait"] = tail
                out.append(ins)
            bb["instructions"] = out
    return j


def _install_wait_split(nc, max_waits=1):
    def to_json_bytes_fixed():
        j = json.loads(mybir.module_to_json_bytes(nc.m))
        return json.dumps(_split_waits_json(j, max_waits)).encode()

    nc.to_json_bytes = to_json_bytes_fixed


def build_program():
    nc = bass.Bass(num_devices=NCORES)

    xa = nc.declare_dram_parameter("xa", [RT, CB, 128, HW, 128], BF16,
                                   isOutput=False)
    wq = nc.declare_dram_parameter("wq", [128, CB, NH, HD], FP8, isOutput=False)
    wk = nc.declare_dram_parameter("wk", [128, CB, NH, HD], FP8, isOutput=False)
    wv = nc.declare_dram_parameter("wv", [128, CB, NH, HD], FP8, isOutput=False)
    wo = nc.declare_dram_parameter("wo", [128, NH, CB, 128], BF16, isOutput=False)
    qb = nc.declare_dram_parameter("qb", [HD, NH], F32, isOutput=False)  # pre-scaled
    kb = nc.declare_dram_parameter("kb", [HD, NH], F32, isOutput=False)
    vb = nc.declare_dram_parameter("vb", [1, C], BF16, isOutput=False)
    lnw = nc.declare_dram_parameter("lnw", [128, CB], F32, isOutput=False)
    lnb = nc.declare_dram_parameter("lnb", [128, CB], F32, isOutput=False)
    bo = nc.declare_dram_parameter("bo", [128, CB], F32, isOutput=False)
    grow = nc.declare_dram_parameter("grow", [1, NS], F32, isOutput=False)
    oha = nc.declare_dram_parameter("oha", [NG, WIN], BF16, isOutput=False)
    ohb = nc.declare_dram_parameter("ohb", [NG, NS], BF16, isOutput=False)
    out = nc.declare_dram_parameter("out", [RT, CB, 128, HW, 128], BF16,
                                    isOutput=True)

    ft_sh = [nc.dram_tensor(f"ft_sh{rt}", [128, CB * 128], FP8)
             for rt in range(RT)]
    ftab = [nc.dram_tensor(f"ftab{rt}", [NCORES * 128, CB * 128], FP8,
                           addr_space="Shared")
            for rt in range(RT)]
    groups = [list(range(NCORES))]
    inv_sqrt_hd = 1.0 / float(np.sqrt(np.float32(HD)))
    BLK = 128 * CB * 128  # elements per core block in ftab

    with tile.TileContext(nc, num_cores=NCORES) as tc:
      with (
        tc.tile_pool(name="singles", bufs=1) as singles,
        tc.tile_pool(name="wop", bufs=1) as wop,
        tc.tile_pool(name="state", bufs=1) as state,
        tc.tile_pool(name="lnp", bufs=1) as lnp,
        tc.tile_pool(name="expp", bufs=9) as expp,
        tc.tile_pool(name="smal", bufs=2) as smal,
        tc.tile_pool(name="mmps", bufs=2, space="PSUM") as mmps,
        tc.tile_pool(name="vps", bufs=1, space="PSUM") as vps,
        tc.tile_pool(name="scps", bufs=3, space="PSUM") as scps,
        tc.tile_pool(name="bcps", bufs=1, space="PSUM") as bcps,
        tc.tile_pool(name="stps", bufs=1, space="PSUM") as stps,
      ):
        # ---------------- preloads ----------------
        wqkv_cm = tc.tile_pool(name="wqkv", bufs=1)
        wqkv = wqkv_cm.__enter__()
        xin_cm = tc.tile_pool(name="xin", bufs=3)
        xin = xin_cm.__enter__()
        ones_col = singles.tile([128, 1], BF16)
        nc.vector.memset(ones_col, 1.0)
        ones_col8 = singles.tile([128, 1], FP8)
        nc.vector.memset(ones_col8, 1.0)
        ones_row = singles.tile([1, 128], BF16)
        nc.vector.memset(ones_row, 1.0)
        ones_row_f = singles.tile([1, 128], F32)
        nc.vector.memset(ones_row_f, 1.0)
        eps_t = singles.tile([1, 1], F32)
        nc.vector.memset(eps_t, EPS * HW * HW)

        wq_t = wqkv.tile([128, CB, NH, HD], FP8)
        wk_t = wqkv.tile([128, CB, NH, HD], FP8)
        wv_t = wqkv.tile([128, CB, NH, HD], FP8)
        wo_t = wop.tile([128, NH, CB, 128], BF16)

        def load_weights():
            # after pool rt1 on the same (sync) queue: rt1 streams at full
            # bandwidth, weights fill the gap before pool rt0 needs it
            nc.sync.dma_start(out=wq_t, in_=wq[:])
            nc.sync.dma_start(out=wk_t, in_=wk[:])
            nc.sync.dma_start(out=wv_t, in_=wv[:])
            nc.sync.dma_start(out=wo_t, in_=wo[:])
        qb_t = singles.tile([128, NH], F32)
        nc.scalar.dma_start(out=qb_t, in_=qb[:])
        kb_t = singles.tile([128, NH], F32)
        nc.scalar.dma_start(out=kb_t, in_=kb[:])
        vb_t = singles.tile([1, C], BF16)
        nc.scalar.dma_start(out=vb_t, in_=vb[:])
        lnw_t = singles.tile([128, CB], F32)
        nc.scalar.dma_start(out=lnw_t, in_=lnw[:])
        lnb_t = singles.tile([128, CB], F32)
        nc.scalar.dma_start(out=lnb_t, in_=lnb[:])
        bo_t = singles.tile([128, CB], F32)
        nc.scalar.dma_start(out=bo_t, in_=bo[:])
        grow_t = singles.tile([1, NS], F32)
        nc.scalar.dma_start(out=grow_t, in_=grow[:])
        oha_t = singles.tile([128, WIN], BF16)
        nc.vector.memset(oha_t, 0.0)
        nc.scalar.dma_start(out=oha_t[:NG, :], in_=oha[:])
        ohb_t = singles.tile([128, NS], BF16)
        nc.vector.memset(ohb_t, 0.0)
        nc.scalar.dma_start(out=ohb_t[:NG, :], in_=ohb[:])

        # gB = gamma*valid broadcast to all partitions (via rank-1 matmul)
        pgb = bcps.tile([128, NS], F32, tag="bc", name="pgb")
        nc.tensor.matmul(pgb, ones_row_f, grow_t, start=True, stop=True)
        gB = singles.tile([128, NS], BF16)
        nc.scalar.activation(out=gB, in_=pgb,
                             func=mybir.ActivationFunctionType.Copy)

        # ---------------- state tiles ----------------
        fsum = state.tile([128, CB, NS], BF16)     # pooled spatial sums
        featn = state.tile([128, CB, NS], FP8)     # LN'd features (own rows)
        qT = state.tile([128, NH, NS], FP8)
        kT = state.tile([128, NH, WIN], FP8)       # cols: own0 own1 haloL haloR
        v_rm = state.tile([128, 4, C], FP8)        # [krow, kt, (h hd)]
        avs = state.tile([128, NH, NS], BF16)      # av / den
        gdT = state.tile([128, CB, NS], BF16)
        wfh = [state.tile([128, CB, 128], FP8, tag=f"wfh{i}", name=f"wfh{i}")
               for i in range(2)]                  # gathered halo feats
        nc.vector.memset(wfh[0], 0.0)              # zero default for edge cores
        nc.vector.memset(wfh[1], 0.0)

        # ---------------- phase functions ----------------
        def pool_tile(rt):
            rsl = slice(rt * 128, (rt + 1) * 128)
            for cb in range(CB):
                xt = xin.tile([128, HW, 128], BF16, tag="xt", name="xt")
                nc.sync.dma_start(out=xt, in_=xa[rt, cb])
                # fold 64 -> 1 on DVE (2x mode: unit stride inner)
                h = HW
                while h > 2:
                    h //= 2
                    nc.vector.tensor_add(out=xt[:, 0:h, :], in0=xt[:, 0:h, :],
                                         in1=xt[:, h:2 * h, :])
                nc.vector.tensor_add(out=fsum[:, cb, rsl], in0=xt[:, 0, :],
                                     in1=xt[:, 1, :])

        def ln_tile(rt):
            rsl = slice(rt * 128, (rt + 1) * 128)
            with tc.high_priority():
                pmu = stps.tile([1, 128], F32, tag="st", name="pmu")
                for cb in range(CB):
                    nc.tensor.matmul(pmu, ones_col, fsum[:, cb, rsl],
                                     start=(cb == 0), stop=(cb == CB - 1))
                sq = lnp.tile([128, CB, 128], BF16, tag="sq", name="sq")
                nc.vector.tensor_mul(out=sq, in0=fsum[:, :, rsl],
                                     in1=fsum[:, :, rsl])
                pss = stps.tile([1, 128], F32, tag="st", name="pss")
                for cb in range(CB):
                    nc.tensor.matmul(pss, ones_col, sq[:, cb, :],
                                     start=(cb == 0), stop=(cb == CB - 1))
                mean_s = lnp.tile([1, 128], F32, tag="mean", name="mean")
                nc.scalar.activation(out=mean_s, in_=pmu,
                                     func=mybir.ActivationFunctionType.Copy,
                                     scale=1.0 / C)
                ms2 = lnp.tile([1, 128], F32, tag="ms2", name="ms2")
                nc.scalar.activation(out=ms2, in_=pss,
                                     func=mybir.ActivationFunctionType.Copy,
                                     scale=1.0 / C)
                var_s = lnp.tile([1, 128], F32, tag="var", name="var")
                nc.vector.tensor_mul(out=var_s, in0=mean_s, in1=mean_s)
                nc.vector.tensor_tensor(out=var_s, in0=ms2, in1=var_s,
                                        op=mybir.AluOpType.subtract)
                std = lnp.tile([1, 128], F32, tag="std", name="std")
                nc.scalar.activation(out=std, in_=var_s,
                                     func=mybir.ActivationFunctionType.Sqrt,
                                     bias=eps_t, scale=1.0)
                rstd = lnp.tile([1, 128], F32, tag="rstd", name="rstd")
                nc.vector.reciprocal(out=rstd, in_=std)
                # broadcast mean/rstd across partitions (PSUM bank shared via
                # copy-then-reuse: bufs=1 on tag "bc")
                pmb = bcps.tile([128, 128], F32, tag="bc", name="pmb")
                nc.tensor.matmul(pmb, ones_row_f, mean_s, start=True, stop=True)
                muB = lnp.tile([128, 128], F32, tag="muB", name="muB")
                nc.scalar.activation(out=muB, in_=pmb,
                                     func=mybir.ActivationFunctionType.Copy)
                prb = bcps.tile([128, 128], F32, tag="bc", name="prb")
                nc.tensor.matmul(prb, ones_row_f, rstd, start=True, stop=True)
                rsB = lnp.tile([128, 128], F32, tag="rsB", name="rsB")
                nc.scalar.activation(out=rsB, in_=prb,
                                     func=mybir.ActivationFunctionType.Copy)
                tmp = lnp.tile([128, 128], F32, tag="tmp", name="tmp")
                for cb in range(CB):
                    nc.vector.tensor_tensor(out=tmp, in0=fsum[:, cb, rsl],
                                            in1=muB,
                                            op=mybir.AluOpType.subtract)
                    nc.vector.tensor_mul(out=tmp, in0=tmp, in1=rsB)
                    nc.scalar.activation(
                        out=featn[:, cb, rsl], in_=tmp,
                        func=mybir.ActivationFunctionType.Identity,
                        bias=lnb_t[:, cb:cb + 1], scale=lnw_t[:, cb:cb + 1])
                nc.gpsimd.dma_start(out=ft_sh[rt][:], in_=featn[:, :, rsl])
                nc.gpsimd.collective_compute(
                    "AllGather", mybir.AluOpType.bypass, replica_groups=groups,
                    ins=[ft_sh[rt][:]], outs=[ftab[rt][:]])

        def qkv_own(rt):
            rsl = slice(rt * 128, (rt + 1) * 128)
            for h in range(NH):
                pq = mmps.tile([128, 128], F32, tag="mm", name="pq")
                for cb in range(CB):
                    nc.tensor.matmul(pq, wq_t[:, cb, h, :], featn[:, cb, rsl],
                                     start=(cb == 0), stop=(cb == CB - 1))
                nc.scalar.activation(
                    out=qT[:, h, rsl], in_=pq,
                    func=mybir.ActivationFunctionType.Identity,
                    bias=qb_t[:, h:h + 1], scale=inv_sqrt_hd)
                pk = mmps.tile([128, 128], F32, tag="mm", name="pk")
                for cb in range(CB):
                    nc.tensor.matmul(pk, wk_t[:, cb, h, :], featn[:, cb, rsl],
                                     start=(cb == 0), stop=(cb == CB - 1))
                nc.scalar.activation(
                    out=kT[:, h, rsl], in_=pk,
                    func=mybir.ActivationFunctionType.Identity,
                    bias=kb_t[:, h:h + 1], scale=1.0)
            for hf in range(2):
                csl = slice(hf * 512, (hf + 1) * 512)
                pv = vps.tile([128, 512], F32, tag="pv", name="pv")
                for cb in range(CB):
                    nc.tensor.matmul(pv, featn[:, cb, rsl],
                                     wv_t[:, cb, 4 * hf:4 * hf + 4, :],
                                     start=(cb == 0), stop=False)
                nc.tensor.matmul(pv, ones_row, vb_t[:, csl],
                                 start=False, stop=True)
                nc.scalar.activation(out=v_rm[:, rt, csl], in_=pv,
                                     func=mybir.ActivationFunctionType.Copy)

        def halo(side):
            # side 0 = haloL (left, from ftab[1], kt=2) ; 1 = haloR (ftab[0], kt=3)
            w = wfh[side]
            pid = nc.sync.partition_id()
            if side == 0:
                off = (pid - 1) * BLK
                tab = ftab[1]
            else:
                off = (pid + 1) * BLK
                tab = ftab[0]
            ap = bass.AP(tensor=tab[:].tensor, offset=off,
                         ap=[[CB * 128, 128], [1, CB * 128]])
            nc.sync.dma_start(out=w, in_=ap, bounds_check="skip_entire_dma")
            ksl = slice((2 + side) * 128, (3 + side) * 128)
            for h in range(NH):
                pk = mmps.tile([128, 128], F32, tag="mm", name="pkh")
                for cb in range(CB):
                    nc.tensor.matmul(pk, wk_t[:, cb, h, :], w[:, cb, :],
                                     start=(cb == 0), stop=(cb == CB - 1))
                nc.scalar.activation(
                    out=kT[:, h, ksl], in_=pk,
                    func=mybir.ActivationFunctionType.Identity,
                    bias=kb_t[:, h:h + 1], scale=1.0)
            for hf in range(2):
                csl = slice(hf * 512, (hf + 1) * 512)
                pv = vps.tile([128, 512], F32, tag="pv", name="pvh")
                for cb in range(CB):
                    nc.tensor.matmul(pv, w[:, cb, :],
                                     wv_t[:, cb, 4 * hf:4 * hf + 4, :],
                                     start=(cb == 0), stop=False)
                nc.tensor.matmul(pv, ones_row, vb_t[:, csl],
                                 start=False, stop=True)
                nc.scalar.activation(out=v_rm[:, 2 + side, csl], in_=pv,
                                     func=mybir.ActivationFunctionType.Copy)

        def attn(qt):
            qsl = slice(qt * 128, (qt + 1) * 128)
            kts = [0, 1, 2] if qt == 0 else [0, 1, 3]
            exs = []
            for h in range(NH):
                ex = expp.tile([128, 3, 128], FP8, tag="ex", name="ex")
                exs.append(ex)
                for i, kt in enumerate(kts):
                    ksl = slice(kt * 128, (kt + 1) * 128)
                    ps = scps.tile([128, 128], F32, tag="sc", name="ps")
                    nc.tensor.matmul(ps, kT[:, h, ksl], qT[:, h, qsl],
                                     start=True, stop=False)
                    nc.tensor.matmul(ps, oha_t[:, ksl], ohb_t[:, qsl],
                                     start=False, stop=True)
                    nc.scalar.activation(
                        out=ex[:, i, :], in_=ps,
                        func=mybir.ActivationFunctionType.Exp)
            for h in range(NH):
                ex = exs[h]
                pden = stps.tile([1, 128], F32, tag="st", name="pden")
                for i in range(3):
                    nc.tensor.matmul(pden, ones_col8, ex[:, i, :],
                                     start=(i == 0), stop=(i == 2))
                rec = smal.tile([1, 128], F32, tag="rec", name="rec")
                nc.vector.reciprocal(out=rec, in_=pden)
                prb = bcps.tile([128, 128], F32, tag="bc", name="prb2")
                nc.tensor.matmul(prb, ones_row_f, rec, start=True, stop=True)
                recB = smal.tile([128, 128], BF16, tag="recB", name="recB")
                nc.scalar.activation(out=recB, in_=prb,
                                     func=mybir.ActivationFunctionType.Copy)
                pav = scps.tile([128, 128], F32, tag="sc", name="pav")
                for i, kt in enumerate(kts):
                    nc.tensor.matmul(pav, v_rm[:, kt, h * HD:(h + 1) * HD],
                                     ex[:, i, :],
                                     start=(i == 0), stop=(i == 2))
                nc.vector.tensor_tensor(out=avs[:, h, qsl], in0=pav, in1=recB,
                                        op=mybir.AluOpType.mult)

        def outproj(qt):
            qsl = slice(qt * 128, (qt + 1) * 128)
            for cb in range(CB):
                pd = mmps.tile([128, 128], F32, tag="mm", name="pd")
                for h in range(NH):
                    nc.tensor.matmul(pd, wo_t[:, h, cb, :], avs[:, h, qsl],
                                     start=(h == 0), stop=(h == NH - 1))
                nc.vector.tensor_scalar(
                    out=gdT[:, cb, qsl], in0=pd, scalar1=bo_t[:, cb:cb + 1],
                    scalar2=None, op0=mybir.AluOpType.add)
                nc.vector.tensor_mul(out=gdT[:, cb, qsl],
                                     in0=gdT[:, cb, qsl], in1=gB[:, qsl])

        def adds(rt, xin3):
            rsl = slice(rt * 128, (rt + 1) * 128)
            for cb in range(CB):
                xt3 = xin3.tile([128, HW, 128], BF16, tag="xt3", name="xt3")
                nc.gpsimd.dma_start(out=xt3, in_=xa[rt, cb])
                nc.vector.tensor_tensor(
                    out=xt3, in0=xt3,
                    in1=gdT[:, cb, rsl][:, None, :].to_broadcast(
                        (128, HW, 128)),
                    op=mybir.AluOpType.add)
                nc.sync.dma_start(out=out[rt, cb], in_=xt3)

        # ---------------- pipeline emission ----------------
        pool_tile(1)
        load_weights()
        ln_tile(1)                  # ... -> AG#1 trigger
        pool_tile(0)
        qkv_own(1)
        ln_tile(0)                  # ... -> AG#0 trigger
        qkv_own(0)
        halo(0)                     # needs AG#1
        attn(0)
        outproj(0)
        halo(1)                     # needs AG#0
        attn(1)
        outproj(1)
        xin_cm.__exit__(None, None, None)
        xin3_cm = tc.tile_pool(name="xin3", bufs=XAB)
        xin3 = xin3_cm.__enter__()
        adds(0, xin3)
        adds(1, xin3)
        xin3_cm.__exit__(None, None, None)
        wqkv_cm.__exit__(None, None, None)

    _install_wait_split(nc)
    return nc


_NC_CACHE = {}


def get_program():
    if "v2" not in _NC_CACHE:
        _NC_CACHE["v2"] = build_program()
    return _NC_CACHE["v2"]


def _groups_ok(bi_sorted):
    counts = np.bincount(bi_sorted, minlength=NG)
    return counts.max() <= GCAP


def prepare_in_maps(x, batch_indices, ln_w, ln_b, in_proj_w, in_proj_b,
                    out_proj_w, out_proj_b, gamma):
    x = np.asarray(x, dtype=np.float32)
    bi_orig = np.asarray(batch_indices).astype(np.int64)
    perm = np.argsort(bi_orig, kind="stable")
    bi = bi_orig[perm]
    assert _groups_ok(bi), "group > 128 rows: fallback path required"
    ln_w = np.asarray(ln_w, np.float32)
    ln_b = np.asarray(ln_b, np.float32)
    ipw = np.asarray(in_proj_w, np.float32)
    ipb = np.asarray(in_proj_b, np.float32)
    opw = np.asarray(out_proj_w, np.float32)
    opb = np.asarray(out_proj_b, np.float32)
    gamma = np.asarray(gamma, np.float32)

    counts = np.bincount(bi, minlength=NG)
    g = np.where(counts[bi] > 1, gamma[0], np.float32(0.0)).astype(np.float32)

    # weights: [ch_in, ch_out] views, p-major over 128-ch blocks
    def wslice(a):  # a: [1024 out, 1024 in] -> [128, CB, NH, HD]
        return np.ascontiguousarray(
            a.T.reshape(CB, 128, NH, HD).transpose(1, 0, 2, 3)
            .astype(ml_dtypes.float8_e4m3))

    wq_h = wslice(ipw[0:C])
    wk_h = wslice(ipw[C:2 * C])
    wv_h = wslice(ipw[2 * C:3 * C])
    # wo: [128 hd, NH, CB, 128 cO]
    wo_h = np.ascontiguousarray(
        opw.T.reshape(NH, HD, CB, 128).transpose(1, 0, 2, 3)
        .astype(ml_dtypes.bfloat16))
    qb_h = np.ascontiguousarray(
        (ipb[0:C] / np.sqrt(np.float32(HD))).reshape(NH, HD).T
        .astype(np.float32))
    kb_h = np.ascontiguousarray(ipb[C:2 * C].reshape(NH, HD).T.astype(np.float32))
    vb_h = np.ascontiguousarray(ipb[2 * C:3 * C].reshape(1, C)
                                .astype(ml_dtypes.bfloat16))
    lnw_h = np.ascontiguousarray(ln_w.reshape(CB, 128).T.astype(np.float32))
    lnb_h = np.ascontiguousarray(ln_b.reshape(CB, 128).T.astype(np.float32))
    bo_h = np.ascontiguousarray(opb.reshape(CB, 128).T.astype(np.float32))

    xs = x[perm]                       # [N, C, 8, 8] sorted
    xs = xs.reshape(N, CB, 128, HW)

    in_maps = []
    for c in range(NCORES):
        r0 = c * NS
        rows = slice(r0, r0 + NS)
        # [RT, CB, 128ch, HW, 128row]: row tile outermost so every chunk DMA
        # reads/writes 16KB-contiguous per partition line
        xc = (xs[rows].reshape(RT, 128, CB, 128, HW)
              .transpose(0, 2, 3, 4, 1))
        # masks over the window [own0 own1 haloL haloR]
        win = np.concatenate([
            np.arange(r0, r0 + NS),
            np.arange(r0 - 128, r0) if c > 0 else np.full(128, -1),
            np.arange(r0 + NS, r0 + NS + 128) if c < NCORES - 1
            else np.full(128, -1),
        ])
        oh_w = np.zeros((NG, WIN), np.float32)
        valid = win >= 0
        oh_w[:, valid] = (bi[win[valid]][None, :]
                          == np.arange(NG)[:, None]).astype(np.float32)
        oha_h = MASK_NEG * oh_w
        oha_h[:, ~valid] = MASK_NEG            # absent halo: mask everywhere
        oh_o = (bi[r0:r0 + NS][None, :] == np.arange(NG)[:, None]).astype(np.float32)
        ohb_h = 1.0 - oh_o
        in_maps.append({
            "xa": np.ascontiguousarray(xc.astype(ml_dtypes.bfloat16)),
            "wq": wq_h, "wk": wk_h, "wv": wv_h, "wo": wo_h,
            "qb": qb_h, "kb": kb_h, "vb": vb_h,
            "lnw": lnw_h, "lnb": lnb_h, "bo": bo_h,
            "grow": np.ascontiguousarray(g[rows].reshape(1, NS)),
            "oha": np.ascontiguousarray(oha_h.astype(ml_dtypes.bfloat16)),
            "ohb": np.ascontiguousarray(ohb_h.astype(ml_dtypes.bfloat16)),
        })
    return in_maps, perm


def assemble(results, perm):
    y_sorted = np.empty((N, C, 8, 8), np.float32)
    for c in range(NCORES):
        o = results[c]["out"].astype(np.float32)  # [RT, CB, 128, HW, 128]
        y_sorted[c * NS:(c + 1) * NS] = (
            o.transpose(0, 4, 1, 2, 3).reshape(NS, C, 8, 8))
    y = np.empty_like(y_sorted)
    y[perm] = y_sorted
    return y


def kernel(**inputs) -> np.ndarray:
    in_maps, perm = prepare_in_maps(**inputs)
    nc = get_program()
    res = run_bass_kernel_spmd(nc, in_maps, list(range(NCORES)), trace=False)
    return assemble(res.results, perm)


# revision 3
# speedup vs baseline: 1.0455x; 1.0455x over previous
"""Trainium2 Bass kernel for nn_BlockCorrelation — v2 (locality rewrite).

Scheme (vs the head-parallel baseline):
  - rows are sorted by group; core c owns the contiguous sorted range
    [c*256, (c+1)*256).  Since every group has <=128 rows, all keys a core's
    queries attend to lie in a +-128-row halo around its range.
  - each core pools + LayerNorms its own rows (bf16 x chunks streamed on the
    sync HWDGE queue, spatial fold 64->1 via DVE 2x-mode tensor adds),
    computes q/k/v for its own rows with FULL heads (no tensor parallelism),
    AllGathers the tiny LN'd feature tiles, reads its two 128-row halos from
    the gathered table with dynamic (partition_id-based) DMA offsets, runs
    block-masked attention locally, and streams y = x + g*deltaT with the
    broadcast on a middle dim so the DVE add runs in 2x mode.
  - NO ReduceScatter; the only cross-core syncs are two 0.25 MB AllGathers
    (one per 128-row tile, pipelined with pooling of the other tile).
  - x layout is [row-tile, cb, ch, hw, row] (row tile OUTERMOST) so every
    chunk DMA is 16KB-contiguous per partition line, pool folds and the gd
    broadcast add both run in DVE 2x mode (inner stride 1, broadcast on the
    middle dim).
  - weights/features/q/k/v/exp run in fp8_e4m3 (numerically free here: the
    rel-err is dominated by the bf16 x passthrough); weights load on the sync
    queue AFTER pool rt1 so the latency-critical first row-tile streams at
    full HBM rate.
  - the block mask is added inside the scores PSUM via the rank-32 one-hot
    matmul; absent halos (edge cores) are memset to zero and masked via
    all-ones -50 one-hot columns (host data), so exp() gives exactly 0.
"""

import json
import sys

if "/opt/trn_rl_repo" not in sys.path:
    sys.path.insert(0, "/opt/trn_rl_repo")

import ml_dtypes
import numpy as np

import concourse.bass as bass
import concourse.mybir as mybir
import concourse.tile as tile
from concourse.bass_utils import run_bass_kernel_spmd

F32 = mybir.dt.float32
BF16 = mybir.dt.bfloat16
FP8 = mybir.dt.float8e4

N, C, HW = 2048, 1024, 64
NH, HD = 8, 128
NG = 32
EPS = 1e-5
NCORES = 8
NS = N // NCORES          # 256 rows per core
CB = C // 128             # 8 channel blocks
RT = 2                    # 128-row tiles per core
WIN = 512                 # key window: [own0, own1, haloL, haloR]
GCAP = 128
MASK_NEG = -50.0
XAB = 7                   # add-pass x prefetch buffers


def _split_waits_json(j, max_waits=1):
    for f in j.get("functions", []):
        for bb in f.get("blocks", []):
            out = []
            for ins in bb.get("instructions", []):
                si = ins.get("sync_info")
                waits = (si or {}).get("on_wait") or []
                if len(waits) > max_waits:
                    head, tail = waits[:-max_waits], waits[-max_waits:]
                    for k, w in enumerate(head):
                        out.append({
                            "name": f"{ins['name']}-wsplit{k}",
                            "opcode": "EventSemaphore",
                            "engine": ins["engine"],
                            "ins": [],
                            "outs": [],
                            "debug": ins.get("debug", 0),
                            "sync_info": {"on_update": [], "on_wait": [w]},
                        })
                    si["on_wait"] = tail
                out.append(ins)
            bb["instructions"] = out
    return j


def _install_wait_split(nc, max_waits=1):
    def to_json_bytes_fixed():
        j = json.loads(mybir.module_to_json_bytes(nc.m))
        return json.dumps(_split_waits_json(j, max_waits)).encode()

    nc.to_json_bytes = to_json_bytes_fixed


def build_program():
    nc = bass.Bass(num_devices=NCORES)

    xa = nc.declare_dram_parameter("xa", [RT, CB, 128, HW, 128], BF16,
                                   isOutput=False)
    wq = nc.declare_dram_parameter("wq", [128, CB, NH, HD], FP8, isOutput=False)
    wk = nc.declare_dram_parameter("wk", [128, CB, NH, HD], FP8, isOutput=False)
    wv = nc.declare_dram_parameter("wv", [128, CB, NH, HD], FP8, isOutput=False)
    wo = nc.declare_dram_parameter("wo", [128, NH, CB, 128], BF16, isOutput=False)
    qb = nc.declare_dram_parameter("qb", [HD, NH], F32, isOutput=False)  # pre-scaled
    kb = nc.declare_dram_parameter("kb", [HD, NH], F32, isOutput=False)
    vb = nc.declare_dram_parameter("vb", [1, C], BF16, isOutput=False)
    lnw = nc.declare_dram_parameter("lnw", [128, CB], F32, isOutput=False)
    lnb = nc.declare_dram_parameter("lnb", [128, CB], F32, isOutput=False)
    bo = nc.declare_dram_parameter("bo", [128, CB], F32, isOutput=False)
    grow = nc.declare_dram_parameter("grow", [1, NS], F32, isOutput=False)
    oha = nc.declare_dram_parameter("oha", [NG, WIN], BF16, isOutput=False)
    ohb = nc.declare_dram_parameter("ohb", [NG, NS], BF16, isOutput=False)
    out = nc.declare_dram_parameter("out", [RT, CB, 128, HW, 128], BF16,
                                    isOutput=True)

    ft_sh = [nc.dram_tensor(f"ft_sh{rt}", [128, CB * 128], FP8)
             for rt in range(RT)]
    ftab = [nc.dram_tensor(f"ftab{rt}", [NCORES * 128, CB * 128], FP8,
                           addr_space="Shared")
            for rt in range(RT)]
    groups = [list(range(NCORES))]
    inv_sqrt_hd = 1.0 / float(np.sqrt(np.float32(HD)))
    BLK = 128 * CB * 128  # elements per core block in ftab

    with tile.TileContext(nc, num_cores=NCORES) as tc:
      with (
        tc.tile_pool(name="singles", bufs=1) as singles,
        tc.tile_pool(name="wop", bufs=1) as wop,
        tc.tile_pool(name="state", bufs=1) as state,
        tc.tile_pool(name="lnp", bufs=1) as lnp,
        tc.tile_pool(name="expp", bufs=9) as expp,
        tc.tile_pool(name="smal", bufs=2) as smal,
        tc.tile_pool(name="mmps", bufs=2, space="PSUM") as mmps,
        tc.tile_pool(name="vps", bufs=1, space="PSUM") as vps,
        tc.tile_pool(name="scps", bufs=3, space="PSUM") as scps,
        tc.tile_pool(name="bcps", bufs=1, space="PSUM") as bcps,
        tc.tile_pool(name="stps", bufs=1, space="PSUM") as stps,
      ):
        # ---------------- preloads ----------------
        wqkv_cm = tc.tile_pool(name="wqkv", bufs=1)
        wqkv = wqkv_cm.__enter__()
        xin_cm = tc.tile_pool(name="xin", bufs=3)
        xin = xin_cm.__enter__()
        ones_col = singles.tile([128, 1], BF16)
        nc.vector.memset(ones_col, 1.0)
        ones_col8 = singles.tile([128, 1], FP8)
        nc.vector.memset(ones_col8, 1.0)
        ones_row = singles.tile([1, 128], BF16)
        nc.vector.memset(ones_row, 1.0)
        ones_row_f = singles.tile([1, 128], F32)
        nc.vector.memset(ones_row_f, 1.0)
        eps_t = singles.tile([1, 1], F32)
        nc.vector.memset(eps_t, EPS * HW * HW)

        wq_t = wqkv.tile([128, CB, NH, HD], FP8)
        wk_t = wqkv.tile([128, CB, NH, HD], FP8)
        wv_t = wqkv.tile([128, CB, NH, HD], FP8)
        wo_t = wop.tile([128, NH, CB, 128], BF16)

        def load_weights():
            # after pool rt1 on the same (sync) queue: rt1 streams at full
            # bandwidth, weights fill the gap before pool rt0 needs it
            nc.sync.dma_start(out=wq_t, in_=wq[:])
            nc.sync.dma_start(out=wk_t, in_=wk[:])
            nc.sync.dma_start(out=wv_t, in_=wv[:])
            nc.sync.dma_start(out=wo_t, in_=wo[:])
        qb_t = singles.tile([128, NH], F32)
        nc.scalar.dma_start(out=qb_t, in_=qb[:])
        kb_t = singles.tile([128, NH], F32)
        nc.scalar.dma_start(out=kb_t, in_=kb[:])
        vb_t = singles.tile([1, C], BF16)
        nc.scalar.dma_start(out=vb_t, in_=vb[:])
        lnw_t = singles.tile([128, CB], F32)
        nc.scalar.dma_start(out=lnw_t, in_=lnw[:])
        lnb_t = singles.tile([128, CB], F32)
        nc.scalar.dma_start(out=lnb_t, in_=lnb[:])
        bo_t = singles.tile([128, CB], F32)
        nc.scalar.dma_start(out=bo_t, in_=bo[:])
        grow_t = singles.tile([1, NS], F32)
        nc.scalar.dma_start(out=grow_t, in_=grow[:])
        oha_t = singles.tile([128, WIN], BF16)
        nc.vector.memset(oha_t, 0.0)
        nc.scalar.dma_start(out=oha_t[:NG, :], in_=oha[:])
        ohb_t = singles.tile([128, NS], BF16)
        nc.vector.memset(ohb_t, 0.0)
        nc.scalar.dma_start(out=ohb_t[:NG, :], in_=ohb[:])

        # gB = gamma*valid broadcast to all partitions (via rank-1 matmul)
        pgb = bcps.tile([128, NS], F32, tag="bc", name="pgb")
        nc.tensor.matmul(pgb, ones_row_f, grow_t, start=True, stop=True)
        gB = singles.tile([128, NS], BF16)
        nc.scalar.activation(out=gB, in_=pgb,
                             func=mybir.ActivationFunctionType.Copy)

        # ---------------- state tiles ----------------
        fsum = state.tile([128, CB, NS], BF16)     # pooled spatial sums
        featn = state.tile([128, CB, NS], FP8)     # LN'd features (own rows)
        qT = state.tile([128, NH, NS], FP8)
        kT = state.tile([128, NH, WIN], FP8)       # cols: own0 own1 haloL haloR
        v_rm = state.tile([128, 4, C], FP8)        # [krow, kt, (h hd)]
        avs = state.tile([128, NH, NS], BF16)      # av / den
        gdT = state.tile([128, CB, NS], BF16)
        wfh = [state.tile([128, CB, 128], FP8, tag=f"wfh{i}", name=f"wfh{i}")
               for i in range(2)]                  # gathered halo feats
        nc.vector.memset(wfh[0], 0.0)              # zero default for edge cores
        nc.vector.memset(wfh[1], 0.0)

        # ---------------- phase functions ----------------
        def pool_tile(rt):
            rsl = slice(rt * 128, (rt + 1) * 128)
            for cb in range(CB):
                xt = xin.tile([128, HW, 128], BF16, tag="xt", name="xt")
                nc.sync.dma_start(out=xt, in_=xa[rt, cb])
                # fold 64 -> 1 on DVE (2x mode: unit stride inner)
                h = HW
                while h > 2:
                    h //= 2
                    nc.vector.tensor_add(out=xt[:, 0:h, :], in0=xt[:, 0:h, :],
                                         in1=xt[:, h:2 * h, :])
                nc.vector.tensor_add(out=fsum[:, cb, rsl], in0=xt[:, 0, :],
                                     in1=xt[:, 1, :])

        def ln_tile(rt):
            rsl = slice(rt * 128, (rt + 1) * 128)
            with tc.high_priority():
                pmu = stps.tile([1, 128], F32, tag="st", name="pmu")
                for cb in range(CB):
                    nc.tensor.matmul(pmu, ones_col, fsum[:, cb, rsl],
                                     start=(cb == 0), stop=(cb == CB - 1))
                sq = lnp.tile([128, CB, 128], BF16, tag="sq", name="sq")
                nc.vector.tensor_mul(out=sq, in0=fsum[:, :, rsl],
                                     in1=fsum[:, :, rsl])
                pss = stps.tile([1, 128], F32, tag="st", name="pss")
                for cb in range(CB):
                    nc.tensor.matmul(pss, ones_col, sq[:, cb, :],
                                     start=(cb == 0), stop=(cb == CB - 1))
                mean_s = lnp.tile([1, 128], F32, tag="mean", name="mean")
                nc.scalar.activation(out=mean_s, in_=pmu,
                                     func=mybir.ActivationFunctionType.Copy,
                                     scale=1.0 / C)
                ms2 = lnp.tile([1, 128], F32, tag="ms2", name="ms2")
                nc.scalar.activation(out=ms2, in_=pss,
                                     func=mybir.ActivationFunctionType.Copy,
                                     scale=1.0 / C)
                var_s = lnp.tile([1, 128], F32, tag="var", name="var")
                nc.vector.tensor_mul(out=var_s, in0=mean_s, in1=mean_s)
                nc.vector.tensor_tensor(out=var_s, in0=ms2, in1=var_s,
                                        op=mybir.AluOpType.subtract)
                std = lnp.tile([1, 128], F32, tag="std", name="std")
                nc.scalar.activation(out=std, in_=var_s,
                                     func=mybir.ActivationFunctionType.Sqrt,
                                     bias=eps_t, scale=1.0)
                rstd = lnp.tile([1, 128], F32, tag="rstd", name="rstd")
                nc.vector.reciprocal(out=rstd, in_=std)
                # broadcast mean/rstd across partitions (PSUM bank shared via
                # copy-then-reuse: bufs=1 on tag "bc")
                pmb = bcps.tile([128, 128], F32, tag="bc", name="pmb")
                nc.tensor.matmul(pmb, ones_row_f, mean_s, start=True, stop=True)
                muB = lnp.tile([128, 128], F32, tag="muB", name="muB")
                nc.scalar.activation(out=muB, in_=pmb,
                                     func=mybir.ActivationFunctionType.Copy)
                prb = bcps.tile([128, 128], F32, tag="bc", name="prb")
                nc.tensor.matmul(prb, ones_row_f, rstd, start=True, stop=True)
                rsB = lnp.tile([128, 128], F32, tag="rsB", name="rsB")
                nc.scalar.activation(out=rsB, in_=prb,
                                     func=mybir.ActivationFunctionType.Copy)
                tmp = lnp.tile([128, 128], F32, tag="tmp", name="tmp")
                for cb in range(CB):
                    nc.vector.tensor_tensor(out=tmp, in0=fsum[:, cb, rsl],
                                            in1=muB,
                                            op=mybir.AluOpType.subtract)
                    nc.vector.tensor_mul(out=tmp, in0=tmp, in1=rsB)
                    nc.scalar.activation(
                        out=featn[:, cb, rsl], in_=tmp,
                        func=mybir.ActivationFunctionType.Identity,
                        bias=lnb_t[:, cb:cb + 1], scale=lnw_t[:, cb:cb + 1])
                nc.gpsimd.dma_start(out=ft_sh[rt][:], in_=featn[:, :, rsl])
                nc.gpsimd.collective_compute(
                    "AllGather", mybir.AluOpType.bypass, replica_groups=groups,
                    ins=[ft_sh[rt][:]], outs=[ftab[rt][:]])

        def qkv_own(rt):
            rsl = slice(rt * 128, (rt + 1) * 128)
            for h in range(NH):
                pq = mmps.tile([128, 128], F32, tag="mm", name="pq")
                for cb in range(CB):
                    nc.tensor.matmul(pq, wq_t[:, cb, h, :], featn[:, cb, rsl],
                                     start=(cb == 0), stop=(cb == CB - 1))
                nc.scalar.activation(
                    out=qT[:, h, rsl], in_=pq,
                    func=mybir.ActivationFunctionType.Identity,
                    bias=qb_t[:, h:h + 1], scale=inv_sqrt_hd)
                pk = mmps.tile([128, 128], F32, tag="mm", name="pk")
                for cb in range(CB):
                    nc.tensor.matmul(pk, wk_t[:, cb, h, :], featn[:, cb, rsl],
                                     start=(cb == 0), stop=(cb == CB - 1))
                nc.scalar.activation(
                    out=kT[:, h, rsl], in_=pk,
                    func=mybir.ActivationFunctionType.Identity,
                    bias=kb_t[:, h:h + 1], scale=1.0)
            for hf in range(2):
                csl = slice(hf * 512, (hf + 1) * 512)
                pv = vps.tile([128, 512], F32, tag="pv", name="pv")
                for cb in range(CB):
                    nc.tensor.matmul(pv, featn[:, cb, rsl],
                                     wv_t[:, cb, 4 * hf:4 * hf + 4, :],
                                     start=(cb == 0), stop=False)
                nc.tensor.matmul(pv, ones_row, vb_t[:, csl],
                                 start=False, stop=True)
                nc.scalar.activation(out=v_rm[:, rt, csl], in_=pv,
                                     func=mybir.ActivationFunctionType.Copy)

        def halo(side):
            # side 0 = haloL (left, from ftab[1], kt=2) ; 1 = haloR (ftab[0], kt=3)
            w = wfh[side]
            pid = nc.sync.partition_id()
            if side == 0:
                off = (pid - 1) * BLK
                tab = ftab[1]
            else:
                off = (pid + 1) * BLK
                tab = ftab[0]
            ap = bass.AP(tensor=tab[:].tensor, offset=off,
                         ap=[[CB * 128, 128], [1, CB * 128]])
            nc.sync.dma_start(out=w, in_=ap, bounds_check="skip_entire_dma")
            ksl = slice((2 + side) * 128, (3 + side) * 128)
            for h in range(NH):
                pk = mmps.tile([128, 128], F32, tag="mm", name="pkh")
                for cb in range(CB):
                    nc.tensor.matmul(pk, wk_t[:, cb, h, :], w[:, cb, :],
                                     start=(cb == 0), stop=(cb == CB - 1))
                nc.scalar.activation(
                    out=kT[:, h, ksl], in_=pk,
                    func=mybir.ActivationFunctionType.Identity,
                    bias=kb_t[:, h:h + 1], scale=1.0)
            for hf in range(2):
                csl = slice(hf * 512, (hf + 1) * 512)
                pv = vps.tile([128, 512], F32, tag="pv", name="pvh")
                for cb in range(CB):
                    nc.tensor.matmul(pv, w[:, cb, :],
                                     wv_t[:, cb, 4 * hf:4 * hf + 4, :],
                                     start=(cb == 0), stop=False)
                nc.tensor.matmul(pv, ones_row, vb_t[:, csl],
                                 start=False, stop=True)
                nc.scalar.activation(out=v_rm[:, 2 + side, csl], in_=pv,
                                     func=mybir.ActivationFunctionType.Copy)

        def attn(qt):
            qsl = slice(qt * 128, (qt + 1) * 128)
            kts = [0, 1, 2] if qt == 0 else [0, 1, 3]
            exs = []
            for h in range(NH):
                ex = expp.tile([128, 3, 128], FP8, tag="ex", name="ex")
                exs.append(ex)
                for i, kt in enumerate(kts):
                    ksl = slice(kt * 128, (kt + 1) * 128)
                    ps = scps.tile([128, 128], F32, tag="sc", name="ps")
                    nc.tensor.matmul(ps, kT[:, h, ksl], qT[:, h, qsl],
                                     start=True, stop=False)
                    nc.tensor.matmul(ps, oha_t[:, ksl], ohb_t[:, qsl],
                                     start=False, stop=True)
                    nc.scalar.activation(
                        out=ex[:, i, :], in_=ps,
                        func=mybir.ActivationFunctionType.Exp)
            for h in range(NH):
                ex = exs[h]
                pden = stps.tile([1, 128], F32, tag="st", name="pden")
                for i in range(3):
                    nc.tensor.matmul(pden, ones_col8, ex[:, i, :],
                                     start=(i == 0), stop=(i == 2))
                rec = smal.tile([1, 128], F32, tag="rec", name="rec")
                nc.vector.reciprocal(out=rec, in_=pden)
                prb = bcps.tile([128, 128], F32, tag="bc", name="prb2")
                nc.tensor.matmul(prb, ones_row_f, rec, start=True, stop=True)
                recB = smal.tile([128, 128], BF16, tag="recB", name="recB")
                nc.scalar.activation(out=recB, in_=prb,
                                     func=mybir.ActivationFunctionType.Copy)
                pav = scps.tile([128, 128], F32, tag="sc", name="pav")
                for i, kt in enumerate(kts):
                    nc.tensor.matmul(pav, v_rm[:, kt, h * HD:(h + 1) * HD],
                                     ex[:, i, :],
                                     start=(i == 0), stop=(i == 2))
                nc.vector.tensor_tensor(out=avs[:, h, qsl], in0=pav, in1=recB,
                                        op=mybir.AluOpType.mult)

        def outproj(qt):
            qsl = slice(qt * 128, (qt + 1) * 128)
            for cb in range(CB):
                pd = mmps.tile([128, 128], F32, tag="mm", name="pd")
                for h in range(NH):
                    nc.tensor.matmul(pd, wo_t[:, h, cb, :], avs[:, h, qsl],
                                     start=(h == 0), stop=(h == NH - 1))
                nc.vector.tensor_scalar(
                    out=gdT[:, cb, qsl], in0=pd, scalar1=bo_t[:, cb:cb + 1],
                    scalar2=None, op0=mybir.AluOpType.add)
                nc.vector.tensor_mul(out=gdT[:, cb, qsl],
                                     in0=gdT[:, cb, qsl], in1=gB[:, qsl])

        def adds(rt, xin3):
            rsl = slice(rt * 128, (rt + 1) * 128)
            for cb in range(CB):
                xt3 = xin3.tile([128, HW, 128], BF16, tag="xt3", name="xt3")
                nc.gpsimd.dma_start(out=xt3, in_=xa[rt, cb])
                nc.vector.tensor_tensor(
                    out=xt3, in0=xt3,
                    in1=gdT[:, cb, rsl][:, None, :].to_broadcast(
                        (128, HW, 128)),
                    op=mybir.AluOpType.add)
                nc.sync.dma_start(out=out[rt, cb], in_=xt3)

        # ---------------- pipeline emission ----------------
        pool_tile(1)
        load_weights()
        ln_tile(1)                  # ... -> AG#1 trigger
        pool_tile(0)
        qkv_own(1)
        ln_tile(0)                  # ... -> AG#0 trigger
        qkv_own(0)
        halo(0)                     # needs AG#1
        attn(0)
        outproj(0)
        halo(1)                     # needs AG#0
        attn(1)
        outproj(1)
        xin_cm.__exit__(None, None, None)
        xin3_cm = tc.tile_pool(name="xin3", bufs=XAB)
        xin3 = xin3_cm.__enter__()
        adds(0, xin3)
        adds(1, xin3)
        xin3_cm.__exit__(None, None, None)
        wqkv_cm.__exit__(None, None, None)

    _install_wait_split(nc)
    return nc


_NC_CACHE = {}


def get_program():
    if "v2" not in _NC_CACHE:
        _NC_CACHE["v2"] = build_program()
    return _NC_CACHE["v2"]


def _groups_ok(bi_sorted):
    counts = np.bincount(bi_sorted, minlength=NG)
    return counts.max() <= GCAP


def prepare_in_maps(x, batch_indices, ln_w, ln_b, in_proj_w, in_proj_b,
                    out_proj_w, out_proj_b, gamma):
    x = np.asarray(x, dtype=np.float32)
    bi_orig = np.asarray(batch_indices).astype(np.int64)
    perm = np.argsort(bi_orig, kind="stable")
    bi = bi_orig[perm]
    assert _groups_ok(bi), "group > 128 rows: fallback path required"
    ln_w = np.asarray(ln_w, np.float32)
    ln_b = np.asarray(ln_b, np.float32)
    ipw = np.asarray(in_proj_w, np.float32)
    ipb = np.asarray(in_proj_b, np.float32)
    opw = np.asarray(out_proj_w, np.float32)
    opb = np.asarray(out_proj_b, np.float32)
    gamma = np.asarray(gamma, np.float32)

    counts = np.bincount(bi, minlength=NG)
    g = np.where(counts[bi] > 1, gamma[0], np.float32(0.0)).astype(np.float32)

    # weights: [ch_in, ch_out] views, p-major over 128-ch blocks
    def wslice(a):  # a: [1024 out, 1024 in] -> [128, CB, NH, HD]
        return np.ascontiguousarray(
            a.T.reshape(CB, 128, NH, HD).transpose(1, 0, 2, 3)
            .astype(ml_dtypes.float8_e4m3))

    wq_h = wslice(ipw[0:C])
    wk_h = wslice(ipw[C:2 * C])
    wv_h = wslice(ipw[2 * C:3 * C])
    # wo: [128 hd, NH, CB, 128 cO]
    wo_h = np.ascontiguousarray(
        opw.T.reshape(NH, HD, CB, 128).transpose(1, 0, 2, 3)
        .astype(ml_dtypes.bfloat16))
    qb_h = np.ascontiguousarray(
        (ipb[0:C] / np.sqrt(np.float32(HD))).reshape(NH, HD).T
        .astype(np.float32))
    kb_h = np.ascontiguousarray(ipb[C:2 * C].reshape(NH, HD).T.astype(np.float32))
    vb_h = np.ascontiguousarray(ipb[2 * C:3 * C].reshape(1, C)
                                .astype(ml_dtypes.bfloat16))
    lnw_h = np.ascontiguousarray(ln_w.reshape(CB, 128).T.astype(np.float32))
    lnb_h = np.ascontiguousarray(ln_b.reshape(CB, 128).T.astype(np.float32))
    bo_h = np.ascontiguousarray(opb.reshape(CB, 128).T.astype(np.float32))

    xs = x[perm]                       # [N, C, 8, 8] sorted
    xs = xs.reshape(N, CB, 128, HW)

    in_maps = []
    for c in range(NCORES):
        r0 = c * NS
        rows = slice(r0, r0 + NS)
        # [RT, CB, 128ch, HW, 128row]: row tile outermost so every chunk DMA
        # reads/writes 16KB-contiguous per partition line
        xc = (xs[rows].reshape(RT, 128, CB, 128, HW)
              .transpose(0, 2, 3, 4, 1))
        # masks over the window [own0 own1 haloL haloR]
        win = np.concatenate([
            np.arange(r0, r0 + NS),
            np.arange(r0 - 128, r0) if c > 0 else np.full(128, -1),
            np.arange(r0 + NS, r0 + NS + 128) if c < NCORES - 1
            else np.full(128, -1),
        ])
        oh_w = np.zeros((NG, WIN), np.float32)
        valid = win >= 0
        oh_w[:, valid] = (bi[win[valid]][None, :]
                          == np.arange(NG)[:, None]).astype(np.float32)
        oha_h = MASK_NEG * oh_w
        oha_h[:, ~valid] = MASK_NEG            # absent halo: mask everywhere
        oh_o = (bi[r0:r0 + NS][None, :] == np.arange(NG)[:, None]).astype(np.float32)
        ohb_h = 1.0 - oh_o
        in_maps.append({
            "xa": np.ascontiguousarray(xc.astype(ml_dtypes.bfloat16)),
            "wq": wq_h, "wk": wk_h, "wv": wv_h, "wo": wo_h,
            "qb": qb_h, "kb": kb_h, "vb": vb_h,
            "lnw": lnw_h, "lnb": lnb_h, "bo": bo_h,
            "grow": np.ascontiguousarray(g[rows].reshape(1, NS)),
            "oha": np.ascontiguousarray(oha_h.astype(ml_dtypes.bfloat16)),
            "ohb": np.ascontiguousarray(ohb_h.astype(ml_dtypes.bfloat16)),
        })
    return in_maps, perm


def assemble(results, perm):
    y_sorted = np.empty((N, C, 8, 8), np.float32)
    for c in range(NCORES):
        o = results[c]["out"].astype(np.float32)  # [RT, CB, 128, HW, 128]
        y_sorted[c * NS:(c + 1) * NS] = (
            o.transpose(0, 4, 1, 2, 3).reshape(NS, C, 8, 8))
    y = np.empty_like(y_sorted)
    y[perm] = y_sorted
    return y


def kernel(**inputs) -> np.ndarray:
    in_maps, perm = prepare_in_maps(**inputs)
    nc = get_program()
    res = run_bass_kernel_spmd(nc, in_maps, list(range(NCORES)), trace=False)
    return assemble(res.results, perm)


# revision 4
# speedup vs baseline: 1.0491x; 1.0035x over previous
"""Trainium2 Bass kernel for nn_BlockCorrelation — v2 (locality rewrite).

Scheme (vs the head-parallel baseline):
  - rows are sorted by group; core c owns the contiguous sorted range
    [c*256, (c+1)*256).  Since every group has <=128 rows, all keys a core's
    queries attend to lie in a +-128-row halo around its range.
  - each core pools + LayerNorms its own rows (bf16 x chunks streamed on the
    sync HWDGE queue, spatial fold 64->1 via DVE 2x-mode tensor adds),
    computes q/k/v for its own rows with FULL heads (no tensor parallelism),
    AllGathers the tiny LN'd feature tiles, reads its two 128-row halos from
    the gathered table with dynamic (partition_id-based) DMA offsets, runs
    block-masked attention locally, and streams y = x + g*deltaT with the
    broadcast on a middle dim so the DVE add runs in 2x mode.
  - NO ReduceScatter; the only cross-core syncs are two 0.25 MB AllGathers
    (one per 128-row tile, pipelined with pooling of the other tile).
  - x layout is [row-tile, cb, ch, hw, row] (row tile OUTERMOST) so every
    chunk DMA is 16KB-contiguous per partition line, pool folds and the gd
    broadcast add both run in DVE 2x mode (inner stride 1, broadcast on the
    middle dim).
  - weights/features/q/k/v/exp run in fp8_e4m3 (numerically free here: the
    rel-err is dominated by the bf16 x passthrough); weights load on the sync
    queue AFTER pool rt1 so the latency-critical first row-tile streams at
    full HBM rate.
  - the block mask is added inside the scores PSUM via the rank-32 one-hot
    matmul; absent halos (edge cores) are memset to zero and masked via
    all-ones -50 one-hot columns (host data), so exp() gives exactly 0.
"""

import json
import sys

if "/opt/trn_rl_repo" not in sys.path:
    sys.path.insert(0, "/opt/trn_rl_repo")

import ml_dtypes
import numpy as np

import concourse.bass as bass
import concourse.mybir as mybir
import concourse.tile as tile
from concourse.bass_utils import run_bass_kernel_spmd

F32 = mybir.dt.float32
BF16 = mybir.dt.bfloat16
FP8 = mybir.dt.float8e4

N, C, HW = 2048, 1024, 64
NH, HD = 8, 128
NG = 32
EPS = 1e-5
NCORES = 8
NS = N // NCORES          # 256 rows per core
CB = C // 128             # 8 channel blocks
RT = 2                    # 128-row tiles per core
WIN = 512                 # key window: [own0, own1, haloL, haloR]
GCAP = 128
MASK_NEG = -50.0
XAB = 7                   # add-pass x prefetch buffers


def _split_waits_json(j, max_waits=1):
    for f in j.get("functions", []):
        for bb in f.get("blocks", []):
            out = []
            for ins in bb.get("instructions", []):
                si = ins.get("sync_info")
                waits = (si or {}).get("on_wait") or []
                if len(waits) > max_waits:
                    head, tail = waits[:-max_waits], waits[-max_waits:]
                    for k, w in enumerate(head):
                        out.append({
                            "name": f"{ins['name']}-wsplit{k}",
                            "opcode": "EventSemaphore",
                            "engine": ins["engine"],
                            "ins": [],
                            "outs": [],
                            "debug": ins.get("debug", 0),
                            "sync_info": {"on_update": [], "on_wait": [w]},
                        })
                    si["on_wait"] = tail
                out.append(ins)
            bb["instructions"] = out
    return j


def _install_wait_split(nc, max_waits=1):
    def to_json_bytes_fixed():
        j = json.loads(mybir.module_to_json_bytes(nc.m))
        return json.dumps(_split_waits_json(j, max_waits)).encode()

    nc.to_json_bytes = to_json_bytes_fixed


def build_program():
    nc = bass.Bass(num_devices=NCORES)

    xa = nc.declare_dram_parameter("xa", [RT, CB, 128, HW, 128], BF16,
                                   isOutput=False)
    wq = nc.declare_dram_parameter("wq", [128, CB, NH, HD], FP8, isOutput=False)
    wk = nc.declare_dram_parameter("wk", [128, CB, NH, HD], FP8, isOutput=False)
    wv = nc.declare_dram_parameter("wv", [128, CB, NH, HD], FP8, isOutput=False)
    wo = nc.declare_dram_parameter("wo", [128, NH, CB, 128], BF16, isOutput=False)
    qb = nc.declare_dram_parameter("qb", [HD, NH], F32, isOutput=False)  # pre-scaled
    kb = nc.declare_dram_parameter("kb", [HD, NH], F32, isOutput=False)
    vb = nc.declare_dram_parameter("vb", [1, C], BF16, isOutput=False)
    lnw = nc.declare_dram_parameter("lnw", [128, CB], F32, isOutput=False)
    lnb = nc.declare_dram_parameter("lnb", [128, CB], F32, isOutput=False)
    bo = nc.declare_dram_parameter("bo", [128, CB], F32, isOutput=False)
    grow = nc.declare_dram_parameter("grow", [1, NS], F32, isOutput=False)
    oha = nc.declare_dram_parameter("oha", [NG, WIN], BF16, isOutput=False)
    ohb = nc.declare_dram_parameter("ohb", [NG, NS], BF16, isOutput=False)
    out = nc.declare_dram_parameter("out", [RT, CB, 128, HW, 128], BF16,
                                    isOutput=True)

    ft_sh = [nc.dram_tensor(f"ft_sh{rt}", [128, CB * 128], FP8)
             for rt in range(RT)]
    ftab = [nc.dram_tensor(f"ftab{rt}", [NCORES * 128, CB * 128], FP8,
                           addr_space="Shared")
            for rt in range(RT)]
    groups = [list(range(NCORES))]
    inv_sqrt_hd = 1.0 / float(np.sqrt(np.float32(HD)))
    BLK = 128 * CB * 128  # elements per core block in ftab

    with tile.TileContext(nc, num_cores=NCORES) as tc:
      with (
        tc.tile_pool(name="singles", bufs=1) as singles,
        tc.tile_pool(name="wop", bufs=1) as wop,
        tc.tile_pool(name="state", bufs=1) as state,
        tc.tile_pool(name="lnp", bufs=1) as lnp,
        tc.tile_pool(name="expp", bufs=9) as expp,
        tc.tile_pool(name="smal", bufs=2) as smal,
        tc.tile_pool(name="mmps", bufs=2, space="PSUM") as mmps,
        tc.tile_pool(name="vps", bufs=1, space="PSUM") as vps,
        tc.tile_pool(name="scps", bufs=3, space="PSUM") as scps,
        tc.tile_pool(name="bcps", bufs=1, space="PSUM") as bcps,
        tc.tile_pool(name="stps", bufs=1, space="PSUM") as stps,
      ):
        # ---------------- preloads ----------------
        wqkv_cm = tc.tile_pool(name="wqkv", bufs=1)
        wqkv = wqkv_cm.__enter__()
        xin_cm = tc.tile_pool(name="xin", bufs=3)
        xin = xin_cm.__enter__()
        ones_col = singles.tile([128, 1], BF16)
        nc.vector.memset(ones_col, 1.0)
        ones_col8 = singles.tile([128, 1], FP8)
        nc.vector.memset(ones_col8, 1.0)
        ones_row = singles.tile([1, 128], BF16)
        nc.vector.memset(ones_row, 1.0)
        ones_row_f = singles.tile([1, 128], F32)
        nc.vector.memset(ones_row_f, 1.0)
        eps_t = singles.tile([1, 1], F32)
        nc.vector.memset(eps_t, EPS * HW * HW)

        wq_t = wqkv.tile([128, CB, NH, HD], FP8)
        wk_t = wqkv.tile([128, CB, NH, HD], FP8)
        wv_t = wqkv.tile([128, CB, NH, HD], FP8)
        wo_t = wop.tile([128, NH, CB, 128], BF16)

        def load_weights():
            # after pool rt1 on the same (sync) queue: rt1 streams at full
            # bandwidth, weights fill the gap before pool rt0 needs it
            nc.sync.dma_start(out=wq_t, in_=wq[:])
            nc.sync.dma_start(out=wk_t, in_=wk[:])
            nc.sync.dma_start(out=wv_t, in_=wv[:])
            nc.sync.dma_start(out=wo_t, in_=wo[:])
        qb_t = singles.tile([128, NH], F32)
        nc.scalar.dma_start(out=qb_t, in_=qb[:])
        kb_t = singles.tile([128, NH], F32)
        nc.scalar.dma_start(out=kb_t, in_=kb[:])
        vb_t = singles.tile([1, C], BF16)
        nc.scalar.dma_start(out=vb_t, in_=vb[:])
        lnw_t = singles.tile([128, CB], F32)
        nc.scalar.dma_start(out=lnw_t, in_=lnw[:])
        lnb_t = singles.tile([128, CB], F32)
        nc.scalar.dma_start(out=lnb_t, in_=lnb[:])
        bo_t = singles.tile([128, CB], F32)
        nc.scalar.dma_start(out=bo_t, in_=bo[:])
        grow_t = singles.tile([1, NS], F32)
        nc.scalar.dma_start(out=grow_t, in_=grow[:])
        oha_t = singles.tile([128, WIN], BF16)
        nc.vector.memset(oha_t, 0.0)
        nc.scalar.dma_start(out=oha_t[:NG, :], in_=oha[:])
        ohb_t = singles.tile([128, NS], BF16)
        nc.vector.memset(ohb_t, 0.0)
        nc.scalar.dma_start(out=ohb_t[:NG, :], in_=ohb[:])

        # gB = gamma*valid broadcast to all partitions (via rank-1 matmul)
        pgb = bcps.tile([128, NS], F32, tag="bc", name="pgb")
        nc.tensor.matmul(pgb, ones_row_f, grow_t, start=True, stop=True)
        gB = singles.tile([128, NS], BF16)
        nc.scalar.activation(out=gB, in_=pgb,
                             func=mybir.ActivationFunctionType.Copy)

        # ---------------- state tiles ----------------
        fsum = state.tile([128, CB, NS], BF16)     # pooled spatial sums
        featn = state.tile([128, CB, NS], FP8)     # LN'd features (own rows)
        qT = state.tile([128, NH, NS], FP8)
        kT = state.tile([128, NH, WIN], FP8)       # cols: own0 own1 haloL haloR
        v_rm = state.tile([128, 4, C], FP8)        # [krow, kt, (h hd)]
        avs = state.tile([128, NH, NS], BF16)      # av / den
        gdT = state.tile([128, CB, NS], BF16)
        wfh = [state.tile([128, CB, 128], FP8, tag=f"wfh{i}", name=f"wfh{i}")
               for i in range(2)]                  # gathered halo feats
        nc.vector.memset(wfh[0], 0.0)              # zero default for edge cores
        nc.vector.memset(wfh[1], 0.0)

        # ---------------- phase functions ----------------
        def pool_tile(rt):
            rsl = slice(rt * 128, (rt + 1) * 128)
            for cb in range(CB):
                xt = xin.tile([128, HW, 128], BF16, tag="xt", name="xt")
                nc.sync.dma_start(out=xt, in_=xa[rt, cb])
                # fold 64 -> 1 on DVE (2x mode: unit stride inner)
                h = HW
                while h > 2:
                    h //= 2
                    nc.vector.tensor_add(out=xt[:, 0:h, :], in0=xt[:, 0:h, :],
                                         in1=xt[:, h:2 * h, :])
                nc.vector.tensor_add(out=fsum[:, cb, rsl], in0=xt[:, 0, :],
                                     in1=xt[:, 1, :])

        def ln_tile(rt):
            rsl = slice(rt * 128, (rt + 1) * 128)
            with tc.high_priority():
                pmu = stps.tile([1, 128], F32, tag="st", name="pmu")
                for cb in range(CB):
                    nc.tensor.matmul(pmu, ones_col, fsum[:, cb, rsl],
                                     start=(cb == 0), stop=(cb == CB - 1))
                sq = lnp.tile([128, CB, 128], BF16, tag="sq", name="sq")
                nc.vector.tensor_mul(out=sq, in0=fsum[:, :, rsl],
                                     in1=fsum[:, :, rsl])
                pss = stps.tile([1, 128], F32, tag="st", name="pss")
                for cb in range(CB):
                    nc.tensor.matmul(pss, ones_col, sq[:, cb, :],
                                     start=(cb == 0), stop=(cb == CB - 1))
                mean_s = lnp.tile([1, 128], F32, tag="mean", name="mean")
                nc.scalar.activation(out=mean_s, in_=pmu,
                                     func=mybir.ActivationFunctionType.Copy,
                                     scale=1.0 / C)
                ms2 = lnp.tile([1, 128], F32, tag="ms2", name="ms2")
                nc.scalar.activation(out=ms2, in_=pss,
                                     func=mybir.ActivationFunctionType.Copy,
                                     scale=1.0 / C)
                var_s = lnp.tile([1, 128], F32, tag="var", name="var")
                nc.vector.tensor_mul(out=var_s, in0=mean_s, in1=mean_s)
                nc.vector.tensor_tensor(out=var_s, in0=ms2, in1=var_s,
                                        op=mybir.AluOpType.subtract)
                std = lnp.tile([1, 128], F32, tag="std", name="std")
                nc.scalar.activation(out=std, in_=var_s,
                                     func=mybir.ActivationFunctionType.Sqrt,
                                     bias=eps_t, scale=1.0)
                rstd = lnp.tile([1, 128], F32, tag="rstd", name="rstd")
                nc.vector.reciprocal(out=rstd, in_=std)
                # broadcast mean/rstd across partitions (PSUM bank shared via
                # copy-then-reuse: bufs=1 on tag "bc")
                pmb = bcps.tile([128, 128], F32, tag="bc", name="pmb")
                nc.tensor.matmul(pmb, ones_row_f, mean_s, start=True, stop=True)
                muB = lnp.tile([128, 128], F32, tag="muB", name="muB")
                nc.scalar.activation(out=muB, in_=pmb,
                                     func=mybir.ActivationFunctionType.Copy)
                prb = bcps.tile([128, 128], F32, tag="bc", name="prb")
                nc.tensor.matmul(prb, ones_row_f, rstd, start=True, stop=True)
                rsB = lnp.tile([128, 128], F32, tag="rsB", name="rsB")
                nc.scalar.activation(out=rsB, in_=prb,
                                     func=mybir.ActivationFunctionType.Copy)
                tmp = lnp.tile([128, 128], F32, tag="tmp", name="tmp")
                for cb in range(CB):
                    nc.vector.tensor_tensor(out=tmp, in0=fsum[:, cb, rsl],
                                            in1=muB,
                                            op=mybir.AluOpType.subtract)
                    nc.vector.tensor_mul(out=tmp, in0=tmp, in1=rsB)
                    nc.scalar.activation(
                        out=featn[:, cb, rsl], in_=tmp,
                        func=mybir.ActivationFunctionType.Identity,
                        bias=lnb_t[:, cb:cb + 1], scale=lnw_t[:, cb:cb + 1])
                nc.gpsimd.dma_start(out=ft_sh[rt][:], in_=featn[:, :, rsl])
                nc.gpsimd.collective_compute(
                    "AllGather", mybir.AluOpType.bypass, replica_groups=groups,
                    ins=[ft_sh[rt][:]], outs=[ftab[rt][:]])

        def qkv_own(rt):
            rsl = slice(rt * 128, (rt + 1) * 128)
            for h in range(NH):
                pq = mmps.tile([128, 128], F32, tag="mm", name="pq")
                for cb in range(CB):
                    nc.tensor.matmul(pq, wq_t[:, cb, h, :], featn[:, cb, rsl],
                                     start=(cb == 0), stop=(cb == CB - 1))
                nc.scalar.activation(
                    out=qT[:, h, rsl], in_=pq,
                    func=mybir.ActivationFunctionType.Identity,
                    bias=qb_t[:, h:h + 1], scale=inv_sqrt_hd)
                pk = mmps.tile([128, 128], F32, tag="mm", name="pk")
                for cb in range(CB):
                    nc.tensor.matmul(pk, wk_t[:, cb, h, :], featn[:, cb, rsl],
                                     start=(cb == 0), stop=(cb == CB - 1))
                nc.scalar.activation(
                    out=kT[:, h, rsl], in_=pk,
                    func=mybir.ActivationFunctionType.Identity,
                    bias=kb_t[:, h:h + 1], scale=1.0)
            for hf in range(2):
                csl = slice(hf * 512, (hf + 1) * 512)
                pv = vps.tile([128, 512], F32, tag="pv", name="pv")
                for cb in range(CB):
                    nc.tensor.matmul(pv, featn[:, cb, rsl],
                                     wv_t[:, cb, 4 * hf:4 * hf + 4, :],
                                     start=(cb == 0), stop=False)
                nc.tensor.matmul(pv, ones_row, vb_t[:, csl],
                                 start=False, stop=True)
                nc.scalar.activation(out=v_rm[:, rt, csl], in_=pv,
                                     func=mybir.ActivationFunctionType.Copy)

        def halo(side):
            # side 0 = haloL (left, from ftab[1], kt=2) ; 1 = haloR (ftab[0], kt=3)
            w = wfh[side]
            pid = nc.sync.partition_id()
            if side == 0:
                off = (pid - 1) * BLK
                tab = ftab[1]
            else:
                off = (pid + 1) * BLK
                tab = ftab[0]
            ap = bass.AP(tensor=tab[:].tensor, offset=off,
                         ap=[[CB * 128, 128], [1, CB * 128]])
            nc.sync.dma_start(out=w, in_=ap, bounds_check="skip_entire_dma")
            ksl = slice((2 + side) * 128, (3 + side) * 128)
            for h in range(NH):
                pk = mmps.tile([128, 128], F32, tag="mm", name="pkh")
                for cb in range(CB):
                    nc.tensor.matmul(pk, wk_t[:, cb, h, :], w[:, cb, :],
                                     start=(cb == 0), stop=(cb == CB - 1))
                nc.scalar.activation(
                    out=kT[:, h, ksl], in_=pk,
                    func=mybir.ActivationFunctionType.Identity,
                    bias=kb_t[:, h:h + 1], scale=1.0)
            for hf in range(2):
                csl = slice(hf * 512, (hf + 1) * 512)
                pv = vps.tile([128, 512], F32, tag="pv", name="pvh")
                for cb in range(CB):
                    nc.tensor.matmul(pv, w[:, cb, :],
                                     wv_t[:, cb, 4 * hf:4 * hf + 4, :],
                                     start=(cb == 0), stop=False)
                nc.tensor.matmul(pv, ones_row, vb_t[:, csl],
                                 start=False, stop=True)
                nc.scalar.activation(out=v_rm[:, 2 + side, csl], in_=pv,
                                     func=mybir.ActivationFunctionType.Copy)

        def attn(qt):
            qsl = slice(qt * 128, (qt + 1) * 128)
            kts = [0, 1, 2] if qt == 0 else [0, 1, 3]
            exs = []
            for h in range(NH):
                ex = expp.tile([128, 3, 128], FP8, tag="ex", name="ex")
                exs.append(ex)
                for i, kt in enumerate(kts):
                    ksl = slice(kt * 128, (kt + 1) * 128)
                    ps = scps.tile([128, 128], F32, tag="sc", name="ps")
                    nc.tensor.matmul(ps, kT[:, h, ksl], qT[:, h, qsl],
                                     start=True, stop=False)
                    nc.tensor.matmul(ps, oha_t[:, ksl], ohb_t[:, qsl],
                                     start=False, stop=True)
                    nc.scalar.activation(
                        out=ex[:, i, :], in_=ps,
                        func=mybir.ActivationFunctionType.Exp)
            for h in range(NH):
                ex = exs[h]
                pden = stps.tile([1, 128], F32, tag="st", name="pden")
                for i in range(3):
                    nc.tensor.matmul(pden, ones_col8, ex[:, i, :],
                                     start=(i == 0), stop=(i == 2))
                rec = smal.tile([1, 128], F32, tag="rec", name="rec")
                nc.vector.reciprocal(out=rec, in_=pden)
                prb = bcps.tile([128, 128], F32, tag="bc", name="prb2")
                nc.tensor.matmul(prb, ones_row_f, rec, start=True, stop=True)
                recB = smal.tile([128, 128], BF16, tag="recB", name="recB")
                nc.scalar.activation(out=recB, in_=prb,
                                     func=mybir.ActivationFunctionType.Copy)
                pav = scps.tile([128, 128], F32, tag="sc", name="pav")
                for i, kt in enumerate(kts):
                    nc.tensor.matmul(pav, v_rm[:, kt, h * HD:(h + 1) * HD],
                                     ex[:, i, :],
                                     start=(i == 0), stop=(i == 2))
                nc.vector.tensor_tensor(out=avs[:, h, qsl], in0=pav, in1=recB,
                                        op=mybir.AluOpType.mult)

        def outproj(qt):
            qsl = slice(qt * 128, (qt + 1) * 128)
            for cb in range(CB):
                pd = mmps.tile([128, 128], F32, tag="mm", name="pd")
                for h in range(NH):
                    nc.tensor.matmul(pd, wo_t[:, h, cb, :], avs[:, h, qsl],
                                     start=(h == 0), stop=(h == NH - 1))
                nc.vector.tensor_scalar(
                    out=gdT[:, cb, qsl], in0=pd, scalar1=bo_t[:, cb:cb + 1],
                    scalar2=None, op0=mybir.AluOpType.add)
                nc.vector.tensor_mul(out=gdT[:, cb, qsl],
                                     in0=gdT[:, cb, qsl], in1=gB[:, qsl])

        def adds(rt, xin3):
            rsl = slice(rt * 128, (rt + 1) * 128)
            for cb in range(CB):
                xt3 = xin3.tile([128, HW, 128], BF16, tag="xt3", name="xt3")
                nc.gpsimd.dma_start(out=xt3, in_=xa[rt, cb])
                nc.vector.tensor_tensor(
                    out=xt3, in0=xt3,
                    in1=gdT[:, cb, rsl][:, None, :].to_broadcast(
                        (128, HW, 128)),
                    op=mybir.AluOpType.add)
                nc.sync.dma_start(out=out[rt, cb], in_=xt3)

        # ---------------- pipeline emission ----------------
        pool_tile(1)
        load_weights()
        ln_tile(1)                  # ... -> AG#1 trigger
        pool_tile(0)
        qkv_own(1)
        ln_tile(0)                  # ... -> AG#0 trigger
        qkv_own(0)
        halo(0)                     # needs AG#1
        attn(0)
        outproj(0)
        xin_cm.__exit__(None, None, None)
        xin3_cm = tc.tile_pool(name="xin3", bufs=XAB)
        xin3 = xin3_cm.__enter__()
        adds(0, xin3)               # before attn(1): the rt0 add TTs must
        halo(1)                     # precede attn(1) on the DVE queue, else
        attn(1)                     # they stall on AG#2 via head-of-line
        outproj(1)
        adds(1, xin3)
        xin3_cm.__exit__(None, None, None)
        wqkv_cm.__exit__(None, None, None)

    _install_wait_split(nc)
    return nc


_NC_CACHE = {}


def get_program():
    if "v2" not in _NC_CACHE:
        _NC_CACHE["v2"] = build_program()
    return _NC_CACHE["v2"]


def _groups_ok(bi_sorted):
    counts = np.bincount(bi_sorted, minlength=NG)
    return counts.max() <= GCAP


def prepare_in_maps(x, batch_indices, ln_w, ln_b, in_proj_w, in_proj_b,
                    out_proj_w, out_proj_b, gamma):
    x = np.asarray(x, dtype=np.float32)
    bi_orig = np.asarray(batch_indices).astype(np.int64)
    perm = np.argsort(bi_orig, kind="stable")
    bi = bi_orig[perm]
    assert _groups_ok(bi), "group > 128 rows: fallback path required"
    ln_w = np.asarray(ln_w, np.float32)
    ln_b = np.asarray(ln_b, np.float32)
    ipw = np.asarray(in_proj_w, np.float32)
    ipb = np.asarray(in_proj_b, np.float32)
    opw = np.asarray(out_proj_w, np.float32)
    opb = np.asarray(out_proj_b, np.float32)
    gamma = np.asarray(gamma, np.float32)

    counts = np.bincount(bi, minlength=NG)
    g = np.where(counts[bi] > 1, gamma[0], np.float32(0.0)).astype(np.float32)

    # weights: [ch_in, ch_out] views, p-major over 128-ch blocks
    def wslice(a):  # a: [1024 out, 1024 in] -> [128, CB, NH, HD]
        return np.ascontiguousarray(
            a.T.reshape(CB, 128, NH, HD).transpose(1, 0, 2, 3)
            .astype(ml_dtypes.float8_e4m3))

    wq_h = wslice(ipw[0:C])
    wk_h = wslice(ipw[C:2 * C])
    wv_h = wslice(ipw[2 * C:3 * C])
    # wo: [128 hd, NH, CB, 128 cO]
    wo_h = np.ascontiguousarray(
        opw.T.reshape(NH, HD, CB, 128).transpose(1, 0, 2, 3)
        .astype(ml_dtypes.bfloat16))
    qb_h = np.ascontiguousarray(
        (ipb[0:C] / np.sqrt(np.float32(HD))).reshape(NH, HD).T
        .astype(np.float32))
    kb_h = np.ascontiguousarray(ipb[C:2 * C].reshape(NH, HD).T.astype(np.float32))
    vb_h = np.ascontiguousarray(ipb[2 * C:3 * C].reshape(1, C)
                                .astype(ml_dtypes.bfloat16))
    lnw_h = np.ascontiguousarray(ln_w.reshape(CB, 128).T.astype(np.float32))
    lnb_h = np.ascontiguousarray(ln_b.reshape(CB, 128).T.astype(np.float32))
    bo_h = np.ascontiguousarray(opb.reshape(CB, 128).T.astype(np.float32))

    xs = x[perm]                       # [N, C, 8, 8] sorted
    xs = xs.reshape(N, CB, 128, HW)

    in_maps = []
    for c in range(NCORES):
        r0 = c * NS
        rows = slice(r0, r0 + NS)
        # [RT, CB, 128ch, HW, 128row]: row tile outermost so every chunk DMA
        # reads/writes 16KB-contiguous per partition line
        xc = (xs[rows].reshape(RT, 128, CB, 128, HW)
              .transpose(0, 2, 3, 4, 1))
        # masks over the window [own0 own1 haloL haloR]
        win = np.concatenate([
            np.arange(r0, r0 + NS),
            np.arange(r0 - 128, r0) if c > 0 else np.full(128, -1),
            np.arange(r0 + NS, r0 + NS + 128) if c < NCORES - 1
            else np.full(128, -1),
        ])
        oh_w = np.zeros((NG, WIN), np.float32)
        valid = win >= 0
        oh_w[:, valid] = (bi[win[valid]][None, :]
                          == np.arange(NG)[:, None]).astype(np.float32)
        oha_h = MASK_NEG * oh_w
        oha_h[:, ~valid] = MASK_NEG            # absent halo: mask everywhere
        oh_o = (bi[r0:r0 + NS][None, :] == np.arange(NG)[:, None]).astype(np.float32)
        ohb_h = 1.0 - oh_o
        in_maps.append({
            "xa": np.ascontiguousarray(xc.astype(ml_dtypes.bfloat16)),
            "wq": wq_h, "wk": wk_h, "wv": wv_h, "wo": wo_h,
            "qb": qb_h, "kb": kb_h, "vb": vb_h,
            "lnw": lnw_h, "lnb": lnb_h, "bo": bo_h,
            "grow": np.ascontiguousarray(g[rows].reshape(1, NS)),
            "oha": np.ascontiguousarray(oha_h.astype(ml_dtypes.bfloat16)),
            "ohb": np.ascontiguousarray(ohb_h.astype(ml_dtypes.bfloat16)),
        })
    return in_maps, perm


def assemble(results, perm):
    y_sorted = np.empty((N, C, 8, 8), np.float32)
    for c in range(NCORES):
        o = results[c]["out"].astype(np.float32)  # [RT, CB, 128, HW, 128]
        y_sorted[c * NS:(c + 1) * NS] = (
            o.transpose(0, 4, 1, 2, 3).reshape(NS, C, 8, 8))
    y = np.empty_like(y_sorted)
    y[perm] = y_sorted
    return y


def kernel(**inputs) -> np.ndarray:
    in_maps, perm = prepare_in_maps(**inputs)
    nc = get_program()
    res = run_bass_kernel_spmd(nc, in_maps, list(range(NCORES)), trace=False)
    return assemble(res.results, perm)
